# revision 15
# baseline (speedup 1.0000x reference)
"""Multi-head attention (B=8, N=1024, DIM=768, H=12) on 8 Trainium2 cores.

Sharding: data-parallel over batch — core b computes batch element b.
Per-core kernel: qkv = x @ w_qkv^T; per-head softmax(q k^T / sqrt(dh)) @ v;
out proj + bias. All matmuls in float32r (TF32-like) mode.

Layout strategy (per core, x_b is [N, D]):
  - host supplies x^T [D, N], w_qkv^T (split qk / v parts), w_proj^T
  - QKV phase: qkT [e, n] feature-major for q,k;  v token-major [n, dh]
    packed per head as lhsT tiles [128, 128] = [v_h | ones] (ones block
    yields softmax denominators for free during attn@V)
  - scoresT[j, i] = k^T(lhsT) x q^T(rhs) -> PSUM; ACT exp (scale folded);
    no max-subtraction (scores ~ N(0,1), max << 80)
  - attn@V: out'[0:64] = unnormalized out^T, out'[64:128] = denom bcast
  - normalize: reciprocal_approx_fast + tensor_tensor mult -> outT [d, n]
  - proj: y[n, f] = outT(lhsT) x wprojT(rhs) + bias (bias tile broadcast
    across partitions via DMA, fused into PSUM evacuation add)
"""

import numpy as np

import concourse.bass as bass
import concourse.mybir as mybir
import concourse.tile as tile
from concourse import bacc
from concourse.alu_op_type import AluOpType
from concourse.bass_utils import run_bass_kernel_spmd

B, N, DIM, H = 8, 1024, 768, 12
DH = DIM // H          # 64
E_QK = 2 * DIM         # 1536
E_V = DIM              # 768
SCALE = DH ** -0.5
NCORES = 8

F32 = mybir.dt.float32
F32R = mybir.dt.float32r

N_TILES = N // 128     # 8
D_CHUNKS = DIM // 128  # 6
QK_TILES = E_QK // 128  # 12
EXP = mybir.ActivationFunctionType.Exp


def build_nc():
    nc = bacc.Bacc("TRN2", target_bir_lowering=False, debug=False,
                   num_devices=NCORES)

    xT_d = nc.dram_tensor("xT", [DIM, N], F32R, kind="ExternalInput")
    wqk_d = nc.dram_tensor("wqkT", [DIM, E_QK], F32R, kind="ExternalInput")
    wv_d = nc.dram_tensor("wvT", [DIM, E_V], F32R, kind="ExternalInput")
    wp_d = nc.dram_tensor("wpT", [DIM, DIM], F32R, kind="ExternalInput")
    bias_d = nc.dram_tensor("b_proj", [DIM], F32, kind="ExternalInput")
    y_d = nc.dram_tensor("y", [N, DIM], F32, kind="ExternalOutput")

    with tile.TileContext(nc) as tc:
        with tc.tile_pool(name="persist", bufs=1) as persist:
            # ---- persistent tiles (live through proj) ----
            outT = persist.tile([128, D_CHUNKS, N], F32R)      # 24K/part
            bias_bc = persist.tile([128, DIM], F32)            # 3K/part

            nc.gpsimd.dma_start(
                out=bias_bc[:],
                in_=bias_d.ap()[None, :].broadcast_to([128, DIM]),
            )

            qkv_sb_ctx = tc.tile_pool(name="qkv_sb", bufs=1)
            qkv_sb = qkv_sb_ctx.__enter__()
            # ---- tiles live until end of attention ----
            qkT = qkv_sb.tile([128, QK_TILES, N], F32R)        # 48K/part
            vp = qkv_sb.tile([128, N_TILES, H, 128], F32R)     # 48K/part

            # ================= Phase A: QKV projections =================
            with (
                tc.tile_pool(name="xpool", bufs=1) as xpool,
                tc.tile_pool(name="psA", bufs=4, space="PSUM") as psA,
            ):
                xT = xpool.tile([128, D_CHUNKS, N], F32R)      # 24K/part
                nc.sync.dma_start(
                    xT[:], xT_d.ap().rearrange("(dc p) n -> p dc n", p=128))

                # ---- V part: v[n, dh] per head + ones block ----
                with tc.tile_pool(name="wvpool", bufs=1) as wvpool:
                    wv = wvpool.tile([128, D_CHUNKS, E_V], F32R)   # 18K/part
                    nc.sync.dma_start(
                        wv[:], wv_d.ap().rearrange("(dc p) f -> p dc f", p=128))
                    for jt in range(N_TILES):
                        for fc, fw in ((0, 512), (512, 256)):
                            ps = psA.tile([128, fw], F32, tag="psA")
                            for dc in range(D_CHUNKS):
                                nc.tensor.matmul(
                                    ps[:],
                                    xT[:, dc, jt * 128:(jt + 1) * 128],
                                    wv[:, dc, fc:fc + fw],
                                    start=(dc == 0), stop=(dc == D_CHUNKS - 1),
                                )
                            h0, nh = fc // DH, fw // DH
                            nc.vector.tensor_copy(
                                vp[:, jt, h0:h0 + nh, 0:DH],
                                ps[:].rearrange("p (h c) -> p h c", c=DH),
                            )
                        # memset on f32r fails walrus ISA check; write the
                        # 1.0f bit pattern through a uint32 view instead
                        nc.vector.memset(
                            vp[:, jt, :, DH:128].bitcast(mybir.dt.uint32),
                            0x3F800000)

                # ---- QK part: qkT [e, n] feature-major ----
                with tc.tile_pool(name="wqkpool", bufs=1) as wqkpool:
                    wqk = wqkpool.tile([128, D_CHUNKS, E_QK], F32R)  # 36K/part
                    nc.sync.dma_start(
                        wqk[:], wqk_d.ap().rearrange("(dc p) e -> p dc e", p=128))
                    for et in range(QK_TILES):
                        for ncn in range(2):
                            ps = psA.tile([128, 512], F32, tag="psA")
                            for dc in range(D_CHUNKS):
                                nc.tensor.matmul(
                                    ps[:],
                                    wqk[:, dc, et * 128:(et + 1) * 128],
                                    xT[:, dc, ncn * 512:(ncn + 1) * 512],
                                    start=(dc == 0), stop=(dc == D_CHUNKS - 1),
                                )
                            nc.vector.tensor_copy(
                                qkT[:, et, ncn * 512:(ncn + 1) * 512], ps[:])

            # ================= Phase B: attention per head =================
            with (
                tc.tile_pool(name="ptpool", bufs=3) as ptpool,
                tc.tile_pool(name="recpool", bufs=2) as recpool,
                tc.tile_pool(name="psS", bufs=3, space="PSUM") as psS,
                tc.tile_pool(name="psO", bufs=1, space="PSUM") as psO,
            ):
                for h in range(H):
                    base = 64 * (h % 2)
                    q_et = h // 2
                    k_et = H // 2 + h // 2
                    ps_o = psO.tile([128, N], F32, tag="ps_o")
                    for jt in range(N_TILES):
                        ps_s = psS.tile([128, N], F32, tag="ps_s")
                        for ic in range(2):
                            nc.tensor.matmul(
                                ps_s[:, ic * 512:(ic + 1) * 512],
                                qkT[base:base + DH, k_et,
                                    jt * 128:(jt + 1) * 128],
                                qkT[base:base + DH, q_et,
                                    ic * 512:(ic + 1) * 512],
                                start=True, stop=True,
                            )
                        pt = ptpool.tile([128, N], F32R, tag="pt")
                        nc.scalar.activation(pt[:], ps_s[:], EXP, scale=SCALE)
                        for ic in range(2):
                            nc.tensor.matmul(
                                ps_o[:, ic * 512:(ic + 1) * 512],
                                vp[:, jt, h, :],
                                pt[:, ic * 512:(ic + 1) * 512],
                                start=(jt == 0), stop=(jt == N_TILES - 1),
                            )
                    # reciprocal_approx_fast only works with SBUF input at
                    # partition base 0 — keep the whole normalize at base 0
                    # and shift-copy the result for odd heads.
                    den = recpool.tile([64, N], F32, tag="den")
                    nc.vector.tensor_copy(den[0:64, :], ps_o[64:128, :])
                    rec = recpool.tile([64, N], F32, tag="rec")
                    nc.vector.reciprocal_approx_fast(rec[0:64, :], den[0:64, :])
                    # out at base 0 or 64; inputs at base 0 (in0 PSUM, in1
                    # SBUF) — DVE handles the out-base shift
                    nc.vector.tensor_tensor(
                        outT[base:base + 64, h // 2, :],
                        ps_o[0:64, :], rec[0:64, :],
                        op=AluOpType.mult)

            qkv_sb_ctx.__exit__(None, None, None)

            # ================= Phase C: output projection =================
            with (
                tc.tile_pool(name="cpool", bufs=1) as cpool,
                tc.tile_pool(name="psC", bufs=4, space="PSUM") as psC,
            ):
                wp = cpool.tile([128, D_CHUNKS, DIM], F32R)        # 18K/part
                nc.sync.dma_start(
                    wp[:], wp_d.ap().rearrange("(dc p) f -> p dc f", p=128))
                yall = cpool.tile([128, N_TILES, DIM], F32)        # 24K/part
                for nt in range(N_TILES):
                    for fc, fw in ((0, 512), (512, 256)):
                        ps = psC.tile([128, fw], F32, tag="psC")
                        for dc in range(D_CHUNKS):
                            nc.tensor.matmul(
                                ps[:],
                                outT[:, dc, nt * 128:(nt + 1) * 128],
                                wp[:, dc, fc:fc + fw],
                                start=(dc == 0), stop=(dc == D_CHUNKS - 1),
                            )
                        nc.vector.tensor_tensor(
                            yall[:, nt, fc:fc + fw], ps[:],
                            bias_bc[:, fc:fc + fw], op=AluOpType.add)
                nc.sync.dma_start(
                    y_d.ap().rearrange("(nt p) f -> p nt f", p=128), yall[:])

    nc.compile()
    return nc


_NC_CACHE = None


def _get_nc():
    global _NC_CACHE
    if _NC_CACHE is None:
        _NC_CACHE = build_nc()
    return _NC_CACHE


_RUNNER_CACHE = None


def _get_runner():
    """Cached jitted shard_map runner over 8 cores (mirrors
    bass2jax.run_bass_via_pjrt, but reusable across calls for timing)."""
    global _RUNNER_CACHE
    if _RUNNER_CACHE is not None:
        return _RUNNER_CACHE
    import jax
    from jax.experimental.shard_map import shard_map
    from jax.sharding import Mesh, PartitionSpec
    from concourse import bass2jax, mybir as _mb

    nc = _get_nc()
    bass2jax.install_neuronx_cc_hook()

    partition_name = (nc.partition_id_tensor.name
                      if nc.partition_id_tensor else None)
    in_names, out_names, out_avals, zero_outs = [], [], [], []
    for alloc in nc.m.functions[0].allocations:
        if not isinstance(alloc, _mb.MemoryLocationSet):
            continue
        name = alloc.memorylocations[0].name
        if alloc.kind == "ExternalInput":
            if name != partition_name:
                in_names.append(name)
        elif alloc.kind == "ExternalOutput":
            out_names.append(name)
            out_avals.append(jax.core.ShapedArray(
                tuple(alloc.tensor_shape), _mb.dt.np(alloc.dtype)))
            zero_outs.append(np.zeros(
                tuple(alloc.tensor_shape), _mb.dt.np(alloc.dtype)))

    n_params = len(in_names)
    all_in_names = in_names + out_names
    if partition_name is not None:
        all_in_names = all_in_names + [partition_name]

    def _body(*args):
        operands = list(args)
        if partition_name is not None:
            operands.append(bass2jax.partition_id_tensor())
        outs = bass2jax._bass_exec_p.bind(
            *operands,
            out_avals=tuple(out_avals),
            in_names=tuple(all_in_names),
            out_names=tuple(out_names),
            lowering_input_output_aliases=(),
            sim_require_finite=True,
            sim_require_nnan=True,
            nc=nc,
        )
        return tuple(outs)

    devices = jax.devices()[:NCORES]
    mesh = Mesh(np.asarray(devices), ("core",))
    n_outs = len(out_names)
    sharded = jax.jit(
        shard_map(
            _body, mesh=mesh,
            in_specs=(PartitionSpec("core"),) * (n_params + n_outs),
            out_specs=(PartitionSpec("core"),) * n_outs,
            check_rep=False,
        ),
        donate_argnums=tuple(range(n_params, n_params + n_outs)),
        keep_unused=True,
    )
    _RUNNER_CACHE = (sharded, in_names, out_names, out_avals, zero_outs)
    return _RUNNER_CACHE


def _prep_inputs(x, w_qkv, w_proj, b_proj):
    x = np.ascontiguousarray(np.asarray(x, dtype=np.float32))
    w_qkv = np.asarray(w_qkv, dtype=np.float32)
    w_proj = np.asarray(w_proj, dtype=np.float32)
    b_proj = np.ascontiguousarray(np.asarray(b_proj, dtype=np.float32))

    xT = np.ascontiguousarray(x.transpose(0, 2, 1))              # [B, D, N]
    wqkT = np.ascontiguousarray(w_qkv[:E_QK].T)                  # [D, 2D]
    wvT = np.ascontiguousarray(w_qkv[E_QK:].T)                   # [D, D]
    wpT = np.ascontiguousarray(w_proj.T)                         # [D, D]
    per_core = {"xT": None, "wqkT": wqkT, "wvT": wvT, "wpT": wpT,
                "b_proj": b_proj}

    def core_map(b):
        m = dict(per_core)
        m["xT"] = xT[b]
        return m

    return [core_map(b) for b in range(NCORES)]


def _run(in_maps):
    sharded, in_names, out_names, out_avals, zero_outs = _get_runner()
    concat_in = [
        np.concatenate([np.asarray(in_maps[c][n]) for c in range(NCORES)],
                       axis=0)
        for n in in_names
    ]
    concat_zeros = [
        np.zeros((NCORES * z.shape[0], *z.shape[1:]), z.dtype)
        for z in zero_outs
    ]
    out_arrs = sharded(*concat_in, *concat_zeros)
    yi = out_names.index("y")
    return np.asarray(out_arrs[yi]).reshape(NCORES, N, DIM)


def kernel(x, w_qkv, w_proj, b_proj):
    in_maps = _prep_inputs(x, w_qkv, w_proj, b_proj)
    return _run(in_maps)


# revision 18
# speedup vs baseline: 6.4836x; 6.4836x over previous
"""Multi-head attention (B=8, N=1024, DIM=768, H=12) on 8 Trainium2 cores.

Sharding: data-parallel over batch — core b computes batch element b.
Per-core kernel: qkv = x @ w_qkv^T; per-head softmax(q k^T / sqrt(dh)) @ v;
out proj + bias. All matmuls in float32r (TF32-like) mode.

Layout strategy (per core, x_b is [N, D]):
  - host supplies x^T [D, N], w_qkv^T (split qk / v parts), w_proj^T
  - QKV phase: qkT [e, n] feature-major for q,k;  v token-major [n, dh]
    packed per head as lhsT tiles [128, 128] = [v_h | ones] (ones block
    yields softmax denominators for free during attn@V)
  - scoresT[j, i] = k^T(lhsT) x q^T(rhs) -> PSUM; ACT exp (scale folded);
    no max-subtraction (scores ~ N(0,1), max << 80)
  - attn@V: out'[0:64] = unnormalized out^T, out'[64:128] = denom bcast
  - normalize: reciprocal_approx_fast + tensor_tensor mult -> outT [d, n]
  - proj: y[n, f] = outT(lhsT) x wprojT(rhs) + bias (bias tile broadcast
    across partitions via DMA, fused into PSUM evacuation add)
"""

import numpy as np

import concourse.bass as bass
import concourse.mybir as mybir
import concourse.tile as tile
from concourse import bacc
from concourse.alu_op_type import AluOpType
from concourse.bass_utils import run_bass_kernel_spmd

B, N, DIM, H = 8, 1024, 768, 12
DH = DIM // H          # 64
E_QK = 2 * DIM         # 1536
E_V = DIM              # 768
SCALE = DH ** -0.5
NCORES = 8

F32 = mybir.dt.float32
F32R = mybir.dt.float32r

N_TILES = N // 128     # 8
D_CHUNKS = DIM // 128  # 6
QK_TILES = E_QK // 128  # 12
EXP = mybir.ActivationFunctionType.Exp


def build_nc(reps=1):
    nc = bacc.Bacc("TRN2", target_bir_lowering=False, debug=False,
                   num_devices=NCORES)

    xT_d = nc.dram_tensor("xT", [DIM, N], F32R, kind="ExternalInput")
    wqk_d = nc.dram_tensor("wqkT", [DIM, E_QK], F32R, kind="ExternalInput")
    wv_d = nc.dram_tensor("wvT", [DIM, E_V], F32R, kind="ExternalInput")
    wp_d = nc.dram_tensor("wpT", [DIM, DIM], F32R, kind="ExternalInput")
    bias_d = nc.dram_tensor("b_proj", [DIM], F32, kind="ExternalInput")
    y_d = nc.dram_tensor("y", [N, DIM], F32, kind="ExternalOutput")

    with tile.TileContext(nc) as tc:
      for _rep in range(reps):
        with tc.tile_pool(name="persist", bufs=1) as persist:
            # ---- persistent tiles (live through proj) ----
            outT = persist.tile([128, D_CHUNKS, N], F32R)      # 24K/part
            bias_bc = persist.tile([128, DIM], F32)            # 3K/part

            nc.gpsimd.dma_start(
                out=bias_bc[:],
                in_=bias_d.ap()[None, :].broadcast_to([128, DIM]),
            )

            with tc.tile_pool(name="qkv_sb", bufs=1) as qkv_sb:
                # ---- tiles live until end of attention ----
                qkT = qkv_sb.tile([128, QK_TILES, N], F32R)        # 48K/part
                vp = qkv_sb.tile([128, N_TILES, H, 128], F32R)     # 48K/part

                # ================= Phase A: QKV projections =================
                with (
                    tc.tile_pool(name="xpool", bufs=1) as xpool,
                    tc.tile_pool(name="psA", bufs=4, space="PSUM") as psA,
                ):
                    xT = xpool.tile([128, D_CHUNKS, N], F32R)      # 24K/part
                    nc.sync.dma_start(
                        xT[:], xT_d.ap().rearrange("(dc p) n -> p dc n", p=128))

                    # ---- V part: v[n, dh] per head + ones block ----
                    with tc.tile_pool(name="wvpool", bufs=1) as wvpool:
                        wv = wvpool.tile([128, D_CHUNKS, E_V], F32R)  # 18K
                        nc.sync.dma_start(
                            wv[:],
                            wv_d.ap().rearrange("(dc p) f -> p dc f", p=128))
                        for jt in range(N_TILES):
                            for fc, fw in ((0, 512), (512, 256)):
                                ps = psA.tile([128, fw], F32, tag="psA")
                                for dc in range(D_CHUNKS):
                                    nc.tensor.matmul(
                                        ps[:],
                                        xT[:, dc, jt * 128:(jt + 1) * 128],
                                        wv[:, dc, fc:fc + fw],
                                        start=(dc == 0),
                                        stop=(dc == D_CHUNKS - 1),
                                    )
                                h0, nh = fc // DH, fw // DH
                                nc.vector.tensor_copy(
                                    vp[:, jt, h0:h0 + nh, 0:DH],
                                    ps[:].rearrange("p (h c) -> p h c", c=DH),
                                )
                            # memset on f32r fails walrus ISA check; write
                            # the 1.0f bit pattern through a uint32 view
                            nc.vector.memset(
                                vp[:, jt, :, DH:128].bitcast(mybir.dt.uint32),
                                0x3F800000)

                    # ---- QK part: qkT [e, n] feature-major; head-pair
                    # order (q-tile, k-tile alternating) so attention can
                    # start as soon as the first pair lands ----
                    with tc.tile_pool(name="wqkpool", bufs=1) as wqkpool:
                        wqk = wqkpool.tile([128, D_CHUNKS, E_QK], F32R)  # 36K
                        nc.sync.dma_start(
                            wqk[:],
                            wqk_d.ap().rearrange("(dc p) e -> p dc e", p=128))
                        et_order = []
                        for i in range(H // 2):
                            et_order += [i, H // 2 + i]
                        for et in et_order:
                            for ncn in range(2):
                                ps = psA.tile([128, 512], F32, tag="psA")
                                for dc in range(D_CHUNKS):
                                    nc.tensor.matmul(
                                        ps[:],
                                        wqk[:, dc, et * 128:(et + 1) * 128],
                                        xT[:, dc, ncn * 512:(ncn + 1) * 512],
                                        start=(dc == 0),
                                        stop=(dc == D_CHUNKS - 1),
                                    )
                                nc.vector.tensor_copy(
                                    qkT[:, et, ncn * 512:(ncn + 1) * 512],
                                    ps[:])

                # ========== Phases B+C: attention + projection ==========
                # psC allocated alongside B pools (2+4+2 = 8 PSUM banks) so
                # projection matmuls fill PE gaps while ACT paces softmax.
                with (
                    tc.tile_pool(name="cpool", bufs=1) as cpool,
                    tc.tile_pool(name="ypool", bufs=2) as ypool,
                    tc.tile_pool(name="psC", bufs=2, space="PSUM") as psC,
                ):
                    wp = cpool.tile([128, D_CHUNKS, DIM], F32R)    # 18K/part
                    nc.sync.dma_start(
                        wp[:], wp_d.ap().rearrange("(dc p) f -> p dc f", p=128))

                    with (
                        tc.tile_pool(name="ptpool", bufs=3) as ptpool,
                        tc.tile_pool(name="recpool", bufs=2) as recpool,
                        tc.tile_pool(name="psS", bufs=2, space="PSUM") as psS,
                        tc.tile_pool(name="psO", bufs=1, space="PSUM") as psO,
                    ):
                        for h in range(H):
                            base = 64 * (h % 2)
                            q_et = h // 2
                            k_et = H // 2 + h // 2
                            ps_o = psO.tile([128, N], F32, tag="ps_o")
                            for jt in range(N_TILES):
                                ps_s = psS.tile([128, N], F32, tag="ps_s")
                                for ic in range(2):
                                    nc.tensor.matmul(
                                        ps_s[:, ic * 512:(ic + 1) * 512],
                                        qkT[base:base + DH, k_et,
                                            jt * 128:(jt + 1) * 128],
                                        qkT[base:base + DH, q_et,
                                            ic * 512:(ic + 1) * 512],
                                        start=True, stop=True,
                                    )
                                pt = ptpool.tile([128, N], F32R, tag="pt")
                                nc.scalar.activation(
                                    pt[:], ps_s[:], EXP, scale=SCALE)
                                for ic in range(2):
                                    nc.tensor.matmul(
                                        ps_o[:, ic * 512:(ic + 1) * 512],
                                        vp[:, jt, h, :],
                                        pt[:, ic * 512:(ic + 1) * 512],
                                        start=(jt == 0),
                                        stop=(jt == N_TILES - 1),
                                    )
                            # reciprocal_approx_fast needs SBUF input at
                            # partition base 0 — normalize at base 0, DVE
                            # handles the out-base shift on the final mult
                            den = recpool.tile([64, N], F32, tag="den")
                            nc.vector.tensor_copy(den[0:64, :],
                                                  ps_o[64:128, :])
                            rec = recpool.tile([64, N], F32, tag="rec")
                            nc.vector.reciprocal_approx_fast(
                                rec[0:64, :], den[0:64, :])
                            nc.vector.tensor_tensor(
                                outT[base:base + 64, h // 2, :],
                                ps_o[0:64, :], rec[0:64, :],
                                op=AluOpType.mult)

                    # ---- projection; emitted last, scheduled into gaps ----
                    for nt in range(N_TILES):
                        yt = ypool.tile([128, DIM], F32, tag="yt")
                        for fc, fw in ((0, 512), (512, 256)):
                            ps = psC.tile([128, fw], F32, tag="psC")
                            for dc in range(D_CHUNKS):
                                nc.tensor.matmul(
                                    ps[:],
                                    outT[:, dc, nt * 128:(nt + 1) * 128],
                                    wp[:, dc, fc:fc + fw],
                                    start=(dc == 0), stop=(dc == D_CHUNKS - 1),
                                )
                            nc.vector.tensor_tensor(
                                yt[:, fc:fc + fw], ps[:],
                                bias_bc[:, fc:fc + fw], op=AluOpType.add)
                        nc.sync.dma_start(
                            y_d.ap().rearrange("(nt p) f -> p nt f",
                                               p=128)[:, nt, :],
                            yt[:])

    nc.compile()
    return nc


_NC_CACHE = None


def _get_nc():
    global _NC_CACHE
    if _NC_CACHE is None:
        _NC_CACHE = build_nc()
    return _NC_CACHE


_RUNNER_CACHE = None


def _get_runner():
    """Cached jitted shard_map runner over 8 cores (mirrors
    bass2jax.run_bass_via_pjrt, but reusable across calls for timing)."""
    global _RUNNER_CACHE
    if _RUNNER_CACHE is not None:
        return _RUNNER_CACHE
    import jax
    from jax.experimental.shard_map import shard_map
    from jax.sharding import Mesh, PartitionSpec
    from concourse import bass2jax, mybir as _mb

    nc = _get_nc()
    bass2jax.install_neuronx_cc_hook()

    partition_name = (nc.partition_id_tensor.name
                      if nc.partition_id_tensor else None)
    in_names, out_names, out_avals, zero_outs = [], [], [], []
    for alloc in nc.m.functions[0].allocations:
        if not isinstance(alloc, _mb.MemoryLocationSet):
            continue
        name = alloc.memorylocations[0].name
        if alloc.kind == "ExternalInput":
            if name != partition_name:
                in_names.append(name)
        elif alloc.kind == "ExternalOutput":
            out_names.append(name)
            out_avals.append(jax.core.ShapedArray(
                tuple(alloc.tensor_shape), _mb.dt.np(alloc.dtype)))
            zero_outs.append(np.zeros(
                tuple(alloc.tensor_shape), _mb.dt.np(alloc.dtype)))

    n_params = len(in_names)
    all_in_names = in_names + out_names
    if partition_name is not None:
        all_in_names = all_in_names + [partition_name]

    def _body(*args):
        operands = list(args)
        if partition_name is not None:
            operands.append(bass2jax.partition_id_tensor())
        outs = bass2jax._bass_exec_p.bind(
            *operands,
            out_avals=tuple(out_avals),
            in_names=tuple(all_in_names),
            out_names=tuple(out_names),
            lowering_input_output_aliases=(),
            sim_require_finite=True,
            sim_require_nnan=True,
            nc=nc,
        )
        return tuple(outs)

    devices = jax.devices()[:NCORES]
    mesh = Mesh(np.asarray(devices), ("core",))
    n_outs = len(out_names)
    sharded = jax.jit(
        shard_map(
            _body, mesh=mesh,
            in_specs=(PartitionSpec("core"),) * (n_params + n_outs),
            out_specs=(PartitionSpec("core"),) * n_outs,
            check_rep=False,
        ),
        donate_argnums=tuple(range(n_params, n_params + n_outs)),
        keep_unused=True,
    )
    _RUNNER_CACHE = (sharded, in_names, out_names, out_avals, zero_outs)
    return _RUNNER_CACHE


def _prep_inputs(x, w_qkv, w_proj, b_proj):
    x = np.ascontiguousarray(np.asarray(x, dtype=np.float32))
    w_qkv = np.asarray(w_qkv, dtype=np.float32)
    w_proj = np.asarray(w_proj, dtype=np.float32)
    b_proj = np.ascontiguousarray(np.asarray(b_proj, dtype=np.float32))

    xT = np.ascontiguousarray(x.transpose(0, 2, 1))              # [B, D, N]
    wqkT = np.ascontiguousarray(w_qkv[:E_QK].T)                  # [D, 2D]
    wvT = np.ascontiguousarray(w_qkv[E_QK:].T)                   # [D, D]
    wpT = np.ascontiguousarray(w_proj.T)                         # [D, D]
    per_core = {"xT": None, "wqkT": wqkT, "wvT": wvT, "wpT": wpT,
                "b_proj": b_proj}

    def core_map(b):
        m = dict(per_core)
        m["xT"] = xT[b]
        return m

    return [core_map(b) for b in range(NCORES)]


def _run(in_maps):
    sharded, in_names, out_names, out_avals, zero_outs = _get_runner()
    concat_in = [
        np.concatenate([np.asarray(in_maps[c][n]) for c in range(NCORES)],
                       axis=0)
        for n in in_names
    ]
    concat_zeros = [
        np.zeros((NCORES * z.shape[0], *z.shape[1:]), z.dtype)
        for z in zero_outs
    ]
    out_arrs = sharded(*concat_in, *concat_zeros)
    yi = out_names.index("y")
    return np.asarray(out_arrs[yi]).reshape(NCORES, N, DIM)


def kernel(x, w_qkv, w_proj, b_proj):
    in_maps = _prep_inputs(x, w_qkv, w_proj, b_proj)
    res = run_bass_kernel_spmd(_get_nc(), in_maps,
                               core_ids=list(range(NCORES)))
    return np.stack([res.results[b]["y"] for b in range(NCORES)], axis=0)
